# revision 12
# baseline (speedup 1.0000x reference)
"""Trainium2 Bass kernel for nn_EncodingInputLayer (embedding_lookup).

Math background
---------------
The reference computes, per batch b:
    v   = one_hot(x[:, :20], 10).reshape(B, 200) @ fc_w.T + fc_b      (B, 9)
    v_map  = broadcast_to(v,      (48, 48, B, 9)).reshape(B, 9, 48, 48)
    o_map  = broadcast_to(others, (48, 48, B, 23)).reshape(B, 23, 48, 48)
    out = all_w conv1x1( concat(oh_w conv1x1 v_map + oh_b,
                                ot_w conv1x1 o_map + ot_b) ) + all_b

The broadcast+raw-reshape *scrambles* batches: flattened, v_map is just
tile(v.flatten(), 48*48).  Working the indexing through (B*9 = 8*2304,
B*23 = 2048*23, 2304 = 48*48) shows batch b's output depends only on
b mod 8:

    out[b] = Map[b % 8],     Map[m] = A1 @ V8m + A2 @ Wm + const
    A1 = all_w[:, :9] @ oh_w, A2 = all_w[:, 9:] @ ot_w
    const = all_w[:, :9] @ oh_b + all_w[:, 9:] @ ot_b + all_b
    V8m[e]  = v.flatten()      [2304*((m+e)%8)  : +2304]          e = 0..8
    Wm[j]   = others.flatten() [(2304*(23m+9j)*256/2304 ...)]      j = 0..22
              (circular windows at offset (256*(23m+9j)) % 47104)

Sharding: pure data parallel over the 8 distinct residues.  Core k gets x
rolled by -256*k batches, which makes its required V8/W windows sit at
*fixed* offsets (the same access patterns on every core -> true SPMD).
Core k computes Map[k] once on-device and DMA-broadcasts it to its 256
output batches (b = k, k+8, ..., 2040).  Host interleaves the 8 outputs.
"""

import numpy as np
from contextlib import ExitStack

import concourse.bass as bass
import concourse.mybir as mybir
import concourse.tile as tile
from concourse import bacc
from concourse.bass_utils import run_bass_kernel_spmd
from concourse.masks import make_identity

F32 = mybir.dt.float32
F32R = mybir.dt.float32r

B = 2048
NF = 43           # flat features per batch
N1 = 20           # one-hot index features
NO = 23           # passthrough features
NCLS = 10         # classes per one-hot
EMB = 9
OUTC = 32
H = W = 48
S = H * W         # 2304
NCORES = 8
BPC = B // NCORES  # 256 output batches per core
VLEN = B * EMB     # 18432 = 8 * 2304
OLEN = B * NO      # 47104


def _emit(nc: bass.Bass):
    x = nc.dram_tensor("x", [B, NF], F32, kind="ExternalInput").ap()
    fc_w = nc.dram_tensor("fc_w", [EMB, N1 * NCLS], F32, kind="ExternalInput").ap()
    fc_b = nc.dram_tensor("fc_b", [EMB], F32, kind="ExternalInput").ap()
    oh_w = nc.dram_tensor("oh_w", [EMB, EMB], F32, kind="ExternalInput").ap()
    oh_b = nc.dram_tensor("oh_b", [EMB], F32, kind="ExternalInput").ap()
    ot_w = nc.dram_tensor("ot_w", [NO, NO], F32, kind="ExternalInput").ap()
    ot_b = nc.dram_tensor("ot_b", [NO], F32, kind="ExternalInput").ap()
    all_w = nc.dram_tensor("all_w", [OUTC, OUTC], F32, kind="ExternalInput").ap()
    all_b = nc.dram_tensor("all_b", [OUTC], F32, kind="ExternalInput").ap()
    out = nc.dram_tensor("out", [BPC, OUTC, S], F32, kind="ExternalOutput").ap()
    v_flat = nc.dram_tensor("v_flat", [VLEN], F32).ap()
    o_flat = nc.dram_tensor("o_flat", [OLEN], F32).ap()

    with ExitStack() as ctx:
        tc = ctx.enter_context(tile.TileContext(nc))
        consts = ctx.enter_context(tc.tile_pool(name="consts", bufs=1))
        xin = ctx.enter_context(tc.tile_pool(name="xin", bufs=4))
        psum_r = ctx.enter_context(tc.tile_pool(name="psum_r", bufs=1, space="PSUM"))
        psum_v = ctx.enter_context(tc.tile_pool(name="psum_v", bufs=2, space="PSUM"))
        psum_m = ctx.enter_context(tc.tile_pool(name="psum_m", bufs=2, space="PSUM"))

        ident = consts.tile([128, 128], F32)
        make_identity(nc, ident)

        # others.flatten() -> contiguous DRAM buffer (DRAM->DRAM DMA)
        nc.sync.dma_start(o_flat.rearrange("(b n) -> b n", n=NO), x[:, N1:NF])

        # x[:, :32] transposed into SBUF via DVE 32x32 block transposes;
        # rows 0..19 are the one-hot index features (rows 20..31 unused)
        xT = consts.tile([32, B], F32)
        for t in range(16):
            xt = xin.tile([128, 32], F32, tag="xt")
            nc.sync.dma_start(xt[:, :], x[128 * t:128 * (t + 1), 0:32])
            for u in range(4):
                nc.vector.transpose(
                    xT[0:32, 128 * t + 32 * u:128 * t + 32 * (u + 1)],
                    xt[32 * u:32 * (u + 1), :],
                )

        # fc_w rearranged: fcw[f, c*9+e] = fc_w[e, f*10+c]
        fcw = consts.tile([N1, NCLS * EMB], F32)
        nc.sync.dma_start(
            fcw.rearrange("f (c e) -> f c e", e=EMB),
            fc_w.rearrange("e (f c) -> f c e", c=NCLS),
        )

        # one-hot masks per class: mask_c[f, b] = (xT[f, b] == c)
        masks = []
        for c in range(NCLS):
            m = consts.tile([N1, B], F32, tag=f"mask{c}")
            eng = nc.vector if c % 2 == 0 else nc.gpsimd
            eng.tensor_scalar(
                out=m[:, :], in0=xT[0:N1, :], scalar1=float(c), scalar2=None,
                op0=mybir.AluOpType.is_equal,
            )
            masks.append(m)

        # v.T (9, 2048) = sum_c (fcw_c).T @ mask_c  + fc_b
        fcb = consts.tile([EMB, 1], F32)
        nc.sync.dma_start(fcb[:, :], fc_b[:, None])
        vT = consts.tile([32, B], F32)
        nc.gpsimd.memset(vT[:, :], 0.0)
        for ch in range(4):
            sl = slice(512 * ch, 512 * (ch + 1))
            pv = psum_v.tile([EMB, 512], F32, tag="v")
            for c in range(NCLS):
                nc.tensor.matmul(
                    pv[:, :],
                    lhsT=fcw[:, EMB * c:EMB * (c + 1)],
                    rhs=masks[c][:, sl],
                    start=(c == 0), stop=(c == NCLS - 1),
                )
            nc.vector.tensor_scalar(
                out=vT[0:EMB, sl], in0=pv[:, :], scalar1=fcb[:, :], scalar2=None,
                op0=mybir.AluOpType.add,
            )

        # transpose v.T back to batch-major via DVE and bounce via DRAM:
        # v_flat[t*1152 + b*9 + e] = v[128t + b, e]
        vr = consts.tile([128, 16 * 32], F32)
        for t in range(16):
            for u in range(4):
                nc.vector.transpose(
                    vr[32 * u:32 * (u + 1), 32 * t:32 * (t + 1)],
                    vT[0:32, 128 * t + 32 * u:128 * t + 32 * (u + 1)],
                )
        nc.sync.dma_start(
            v_flat.rearrange("(t b e) -> b t e", t=16, e=EMB),
            vr.rearrange("b (t e) -> b t e", e=32)[:, :, 0:EMB],
        )

        # rhs (33, 2304): rows 0..8 = V8 windows, rows 9..31 = W windows,
        # row 32 = ones (carries the const term).
        rhs = consts.tile([EMB + NO + 1, S], F32)
        nc.sync.dma_start(rhs[0:8, :], v_flat.rearrange("(r s) -> r s", s=S))
        nc.sync.dma_start(rhs[8:9, :], v_flat[0:S][None, :])
        # W row j lives at offset (2304*j) % 47104; j=0..19 contiguous,
        # j=20 wraps, j=21..22 restart at 1280.
        nc.sync.dma_start(rhs[9:29, :], o_flat[0:20 * S].rearrange("(j s) -> j s", s=S))
        nc.sync.dma_start(rhs[29:30, 0:1024], o_flat[20 * S:OLEN][None, :])
        nc.sync.dma_start(rhs[29:30, 1024:S], o_flat[0:1280][None, :])
        nc.sync.dma_start(rhs[30:32, :], o_flat[1280:1280 + 2 * S].rearrange("(j s) -> j s", s=S))
        nc.gpsimd.memset(rhs[32:33, :], 1.0)

        # lhsT (33, 32), replicated 4x along free dim -> (33, 128):
        # rows 0..8 = A1.T, rows 9..31 = A2.T, row 32 = const.
        awT = consts.tile([OUTC, OUTC], F32)
        nc.sync.dma_start(awT[:, :], all_w.rearrange("c i -> i c"))
        awT2 = consts.tile([NO, OUTC], F32)
        nc.sync.dma_start(awT2[:, :], all_w.rearrange("c i -> i c")[EMB:OUTC, :])
        ohw = consts.tile([EMB, EMB], F32)
        nc.sync.dma_start(ohw[:, :], oh_w)
        otw = consts.tile([NO, NO], F32)
        nc.sync.dma_start(otw[:, :], ot_w)
        bvec = consts.tile([OUTC, 1], F32)
        nc.sync.dma_start(bvec[0:EMB, :], oh_b[:, None])
        nc.sync.dma_start(bvec[EMB:OUTC, :], ot_b[:, None])
        allb = consts.tile([1, OUTC], F32)
        nc.sync.dma_start(allb[:, :], all_b[None, :])

        # one PSUM tile, each matmul in its own 2KB bank (free offsets 0/512/1024)
        pl = psum_r.tile([NO, 1536], F32, tag="t2")
        nc.tensor.matmul(pl[0:EMB, 0:OUTC], lhsT=ohw[:, :], rhs=awT[0:EMB, :],
                         start=True, stop=True)
        nc.tensor.matmul(pl[0:NO, 512:512 + OUTC], lhsT=otw[:, :], rhs=awT2[:, :],
                         start=True, stop=True)
        nc.tensor.matmul(pl[0:1, 1024:1024 + OUTC], lhsT=bvec[:, :], rhs=awT[:, :],
                         start=True, stop=True)
        # compute engines cannot write partition bases 9/32 -> bounce the
        # pieces through base-0 SBUF temps and assemble lhsT with DMAs
        tA = consts.tile([EMB, OUTC], F32)
        nc.vector.tensor_copy(tA[:, :], pl[0:EMB, 0:OUTC])
        tB = consts.tile([NO, OUTC], F32)
        nc.vector.tensor_copy(tB[:, :], pl[0:NO, 512:512 + OUTC])
        tC = consts.tile([1, OUTC], F32)
        nc.vector.tensor_add(tC[:, :], pl[0:1, 1024:1024 + OUTC], allb[:, :])
        lhsT = consts.tile([EMB + NO + 1, 4 * OUTC], F32)
        for r in range(4):
            sl = slice(OUTC * r, OUTC * (r + 1))
            nc.sync.dma_start(lhsT[0:EMB, sl], tA[:, :])
            nc.sync.dma_start(lhsT[EMB:EMB + NO, sl], tB[:, :])
            nc.sync.dma_start(lhsT[EMB + NO:, sl], tC[:, :])

        # Map matmul: (33, 128).T @ (33, 2304) -> psum (128, 2304) in 512-col
        # chunks; partitions hold 4 batch-replicas of the 32 channels.
        # map16 holds 4 additional spatial replicas -> one DMA covers 16
        # output batches.
        map16 = consts.tile([128, 4 * S], F32)
        for ch in range(5):
            sz = 512 if ch < 4 else 256
            pm = psum_m.tile([128, 512], F32, tag="m")
            nc.tensor.matmul(
                pm[:, 0:sz],
                lhsT=lhsT[:, :],
                rhs=rhs[:, 512 * ch:512 * ch + sz],
                start=True, stop=True,
            )
            for r in range(4):
                nc.vector.tensor_copy(
                    map16[:, r * S + 512 * ch: r * S + 512 * ch + sz], pm[:, 0:sz]
                )

        # Output: 16 DMAs x 4.7MB, alternating the two HWDGE rings.
        src = map16.rearrange("p (a s) -> p a s", a=4)
        for g in range(16):
            dst = out[16 * g:16 * (g + 1)].rearrange("(a l) c s -> (l c) a s", a=4)
            eng = nc.sync if g % 2 == 0 else nc.scalar
            eng.dma_start(dst, src)

    return nc


_NC_CACHE: dict = {}


def _get_nc():
    if "nc" not in _NC_CACHE:
        nc = bacc.Bacc("TRN2", target_bir_lowering=False, debug=False,
                       num_devices=NCORES)
        _emit(nc)
        nc.compile()
        _NC_CACHE["nc"] = nc
    return _NC_CACHE["nc"]


def kernel(x, fc_w, fc_b, oh_w, oh_b, ot_w, ot_b, all_w, all_b):
    nc = _get_nc()
    xf = np.ascontiguousarray(np.asarray(x, dtype=np.float32).reshape(B, NF))
    params = {
        "fc_w": np.ascontiguousarray(fc_w, dtype=np.float32),
        "fc_b": np.ascontiguousarray(fc_b, dtype=np.float32),
        "oh_w": np.ascontiguousarray(oh_w, dtype=np.float32),
        "oh_b": np.ascontiguousarray(oh_b, dtype=np.float32),
        "ot_w": np.ascontiguousarray(ot_w, dtype=np.float32),
        "ot_b": np.ascontiguousarray(ot_b, dtype=np.float32),
        "all_w": np.ascontiguousarray(all_w, dtype=np.float32),
        "all_b": np.ascontiguousarray(all_b, dtype=np.float32),
    }
    in_maps = [
        {"x": np.ascontiguousarray(np.roll(xf, -BPC * k, axis=0)), **params}
        for k in range(NCORES)
    ]
    res = run_bass_kernel_spmd(nc, in_maps, list(range(NCORES)))
    full = np.empty((B, OUTC, H, W), dtype=np.float32)
    for k in range(NCORES):
        full[k::NCORES] = res.results[k]["out"].reshape(BPC, OUTC, H, W)
    return full


# revision 17
# speedup vs baseline: 1.5638x; 1.5638x over previous
"""Trainium2 Bass kernel for nn_EncodingInputLayer (embedding_lookup).

Math background
---------------
The reference computes, per batch b:
    v   = one_hot(x[:, :20], 10).reshape(B, 200) @ fc_w.T + fc_b      (B, 9)
    v_map  = broadcast_to(v,      (48, 48, B, 9)).reshape(B, 9, 48, 48)
    o_map  = broadcast_to(others, (48, 48, B, 23)).reshape(B, 23, 48, 48)
    out = all_w conv1x1( concat(oh_w conv1x1 v_map + oh_b,
                                ot_w conv1x1 o_map + ot_b) ) + all_b

The broadcast+raw-reshape *scrambles* batches: flattened, v_map is just
tile(v.flatten(), 48*48).  Working the indexing through (B*9 = 8*2304,
B*23 = 2048*23, 2304 = 48*48) shows batch b's output depends only on
b mod 8:

    out[b] = Map[b % 8],     Map[m] = A1 @ V8m + A2 @ Wm + const
    A1 = all_w[:, :9] @ oh_w, A2 = all_w[:, 9:] @ ot_w
    const = all_w[:, :9] @ oh_b + all_w[:, 9:] @ ot_b + all_b
    V8m[e]  = v.flatten()      [2304*((m+e)%8)  : +2304]          e = 0..8
    Wm[j]   = others.flatten() [(2304*(23m+9j)*256/2304 ...)]      j = 0..22
              (circular windows at offset (256*(23m+9j)) % 47104)

Sharding: pure data parallel over the 8 distinct residues.  Core k gets x
rolled by -256*k batches, which makes its required V8/W windows sit at
*fixed* offsets (the same access patterns on every core -> true SPMD).
Core k computes Map[k] once on-device and DMA-broadcasts it to its 256
output batches (b = k, k+8, ..., 2040).  Host interleaves the 8 outputs.
"""

import numpy as np
from contextlib import ExitStack

import concourse.bass as bass
import concourse.mybir as mybir
import concourse.tile as tile
from concourse import bacc
from concourse.bass_utils import run_bass_kernel_spmd
from concourse.masks import make_identity

F32 = mybir.dt.float32
F32R = mybir.dt.float32r

B = 2048
NF = 43           # flat features per batch
N1 = 20           # one-hot index features
NO = 23           # passthrough features
NCLS = 10         # classes per one-hot
EMB = 9
OUTC = 32
H = W = 48
S = H * W         # 2304
NCORES = 8
BPC = B // NCORES  # 256 output batches per core
VLEN = B * EMB     # 18432 = 8 * 2304
OLEN = B * NO      # 47104


def _emit(nc: bass.Bass):
    x = nc.dram_tensor("x", [B, NF], F32, kind="ExternalInput").ap()
    fc_w = nc.dram_tensor("fc_w", [EMB, N1 * NCLS], F32, kind="ExternalInput").ap()
    fc_b = nc.dram_tensor("fc_b", [EMB], F32, kind="ExternalInput").ap()
    oh_w = nc.dram_tensor("oh_w", [EMB, EMB], F32, kind="ExternalInput").ap()
    oh_b = nc.dram_tensor("oh_b", [EMB], F32, kind="ExternalInput").ap()
    ot_w = nc.dram_tensor("ot_w", [NO, NO], F32, kind="ExternalInput").ap()
    ot_b = nc.dram_tensor("ot_b", [NO], F32, kind="ExternalInput").ap()
    all_w = nc.dram_tensor("all_w", [OUTC, OUTC], F32, kind="ExternalInput").ap()
    all_b = nc.dram_tensor("all_b", [OUTC], F32, kind="ExternalInput").ap()
    out = nc.dram_tensor("out", [BPC, OUTC, S], F32, kind="ExternalOutput").ap()
    v_flat = nc.dram_tensor("v_flat", [VLEN], F32R).ap()
    o_flat = nc.dram_tensor("o_flat", [OLEN], F32R).ap()

    with ExitStack() as ctx:
        tc = ctx.enter_context(tile.TileContext(nc))
        consts = ctx.enter_context(tc.tile_pool(name="consts", bufs=1))
        psum_r = ctx.enter_context(tc.tile_pool(name="psum_r", bufs=1, space="PSUM"))
        psum_v = ctx.enter_context(tc.tile_pool(name="psum_v", bufs=2, space="PSUM"))
        psum_m = ctx.enter_context(tc.tile_pool(name="psum_m", bufs=2, space="PSUM"))

        # others.flatten() -> contiguous DRAM buffer (DRAM->DRAM DMA).
        # Values are small integers, so the f32->f32r bitcast is exact.
        nc.sync.dma_start(
            o_flat.rearrange("(b n) -> b n", n=NO), x[:, N1:NF].bitcast(F32R)
        )

        # One DMA loads x[:, :32] as 16 stacked (128, 32) tiles.
        xbig = consts.tile([128, 16 * 32], F32)
        nc.sync.dma_start(
            xbig.rearrange("p (t j) -> p t j", j=32),
            x.rearrange("(t p) j -> p t j", p=128)[:, :, 0:32],
        )

        # Transpose to xT4 (128, 512): partition 32*g + j holds feature j of
        # batch group g (512 batches per group); feature rows 20..31 are the
        # passthrough features (masked out by zero weight rows below).
        xT4 = consts.tile([128, 512], F32)
        for t in range(16):
            g, tm = t // 4, t % 4
            for u in range(4):
                nc.vector.transpose(
                    xT4[32 * g:32 * (g + 1), 128 * tm + 32 * u:128 * tm + 32 * (u + 1)],
                    xbig[32 * u:32 * (u + 1), 32 * t:32 * (t + 1)],
                )

        # fcw4 (128, 90) f32r: fcw4[32g + f, c*9 + e] = fc_w[e, f*10 + c],
        # replicated across the 4 groups, zero on feature rows 20..31.
        fcw4 = consts.tile([128, NCLS * EMB], F32R)
        nc.vector.memset(fcw4.bitcast(F32)[:, :], 0.0)
        for g in range(4):
            nc.sync.dma_start(
                fcw4[32 * g:32 * g + N1, :].rearrange("f (c e) -> f c e", e=EMB),
                fc_w.rearrange("e (f c) -> f c e", c=NCLS).bitcast(F32R),
            )

        # one-hot masks per class: mask_c[32g + f, b'] = (x[512g + b', f] == c)
        masks = []
        for c in range(NCLS):
            m = consts.tile([128, 512], F32R, tag=f"mask{c}")
            nc.vector.tensor_scalar(
                out=m[:, :], in0=xT4[:, :], scalar1=float(c), scalar2=None,
                op0=mybir.AluOpType.is_equal,
            )
            masks.append(m)

        # v.T (9, 2048) = sum_c (fcw_c).T @ mask_c + fc_b, one matmul group
        # per 512-batch group at PE tile position (32g, 0)
        fcb = consts.tile([EMB, 1], F32)
        nc.sync.dma_start(fcb[:, :], fc_b[:, None])
        vT = consts.tile([32, B], F32R)
        nc.vector.memset(vT.bitcast(F32)[:, :], 0.0)
        for g in range(4):
            sl = slice(512 * g, 512 * (g + 1))
            pv = psum_v.tile([EMB, 512], F32, tag="v")
            for c in range(NCLS):
                nc.tensor.matmul(
                    pv[:, :],
                    lhsT=fcw4[32 * g:32 * g + N1, EMB * c:EMB * (c + 1)],
                    rhs=masks[c][32 * g:32 * g + N1, :],
                    start=(c == 0), stop=(c == NCLS - 1),
                    tile_position=(32 * g, 0),
                )
            nc.vector.tensor_scalar(
                out=vT[0:EMB, sl], in0=pv[:, :], scalar1=fcb[:, :], scalar2=None,
                op0=mybir.AluOpType.add,
            )

        # transpose v.T back to batch-major via DVE and bounce via DRAM:
        # v_flat[t*1152 + b*9 + e] = v[128t + b, e]
        vr = consts.tile([128, 16 * 32], F32R)
        for t in range(16):
            for u in range(4):
                nc.vector.transpose(
                    vr.bitcast(F32)[32 * u:32 * (u + 1), 32 * t:32 * (t + 1)],
                    vT.bitcast(F32)[0:32, 128 * t + 32 * u:128 * t + 32 * (u + 1)],
                )
        nc.sync.dma_start(
            v_flat.rearrange("(t b e) -> b t e", t=16, e=EMB),
            vr.rearrange("b (t e) -> b t e", e=32)[:, :, 0:EMB],
        )

        # rhs (33, 2304) f32r: rows 0..8 = V8 windows, rows 9..31 = W windows,
        # row 32 = ones (carries the const term).
        rhs = consts.tile([EMB + NO + 1, S], F32R)
        nc.sync.dma_start(rhs[0:8, :], v_flat.rearrange("(r s) -> r s", s=S))
        nc.sync.dma_start(rhs[8:9, :], v_flat[0:S][None, :])
        # W row j lives at offset (2304*j) % 47104; j=0..19 contiguous,
        # j=20 wraps, j=21..22 restart at 1280.
        nc.sync.dma_start(rhs[9:29, :], o_flat[0:20 * S].rearrange("(j s) -> j s", s=S))
        nc.sync.dma_start(rhs[29:30, 0:1024], o_flat[20 * S:OLEN][None, :])
        nc.sync.dma_start(rhs[29:30, 1024:S], o_flat[0:1280][None, :])
        nc.sync.dma_start(rhs[30:32, :], o_flat[1280:1280 + 2 * S].rearrange("(j s) -> j s", s=S))
        nc.vector.memset(rhs.bitcast(F32)[32:33, :], 1.0)

        # lhsT (33, 32) f32r, replicated 4x along free dim -> (33, 128):
        # rows 0..8 = A1.T, rows 9..31 = A2.T, row 32 = const.
        awT = consts.tile([OUTC, OUTC], F32)
        nc.sync.dma_start(awT[:, :], all_w.rearrange("c i -> i c"))
        awT2 = consts.tile([NO, OUTC], F32)
        nc.sync.dma_start(awT2[:, :], all_w.rearrange("c i -> i c")[EMB:OUTC, :])
        ohw = consts.tile([EMB, EMB], F32)
        nc.sync.dma_start(ohw[:, :], oh_w)
        otw = consts.tile([NO, NO], F32)
        nc.sync.dma_start(otw[:, :], ot_w)
        bvec = consts.tile([OUTC, 1], F32)
        nc.sync.dma_start(bvec[0:EMB, :], oh_b[:, None])
        nc.sync.dma_start(bvec[EMB:OUTC, :], ot_b[:, None])
        allb = consts.tile([1, OUTC], F32)
        nc.sync.dma_start(allb[:, :], all_b[None, :])

        # one PSUM tile, each matmul in its own 2KB bank (free offsets 0/512/1024)
        pl = psum_r.tile([NO, 1536], F32, tag="t2")
        nc.tensor.matmul(pl[0:EMB, 0:OUTC], lhsT=ohw[:, :], rhs=awT[0:EMB, :],
                         start=True, stop=True)
        nc.tensor.matmul(pl[0:NO, 512:512 + OUTC], lhsT=otw[:, :], rhs=awT2[:, :],
                         start=True, stop=True)
        nc.tensor.matmul(pl[0:1, 1024:1024 + OUTC], lhsT=bvec[:, :], rhs=awT[:, :],
                         start=True, stop=True)
        # compute engines cannot write partition bases 9/32 -> bounce the
        # pieces through base-0 SBUF temps and assemble lhsT with DMAs
        tA = consts.tile([EMB, OUTC], F32R)
        nc.vector.tensor_copy(tA[:, :], pl[0:EMB, 0:OUTC])
        tB = consts.tile([NO, OUTC], F32R)
        nc.vector.tensor_copy(tB[:, :], pl[0:NO, 512:512 + OUTC])
        tC = consts.tile([1, OUTC], F32R)
        nc.vector.tensor_add(tC[:, :], pl[0:1, 1024:1024 + OUTC], allb[:, :])
        lhsT = consts.tile([EMB + NO + 1, 4 * OUTC], F32R)
        for r in range(4):
            sl = slice(OUTC * r, OUTC * (r + 1))
            nc.sync.dma_start(lhsT[0:EMB, sl], tA[:, :])
            nc.sync.dma_start(lhsT[EMB:EMB + NO, sl], tB[:, :])
            nc.sync.dma_start(lhsT[EMB + NO:, sl], tC[:, :])

        # Map matmul: (33, 128).T @ (33, 2304) -> psum (128, 2304) in 512-col
        # chunks; partitions hold 4 batch-replicas of the 32 channels.
        # map16 holds 4 additional spatial replicas -> one DMA covers 16
        # output batches.
        map16 = consts.tile([128, 4 * S], F32)
        for ch in range(5):
            sz = 512 if ch < 4 else 256
            pm = psum_m.tile([128, 512], F32, tag="m")
            nc.tensor.matmul(
                pm[:, 0:sz],
                lhsT=lhsT[:, :],
                rhs=rhs[:, 512 * ch:512 * ch + sz],
                start=True, stop=True,
            )
            for r in range(4):
                nc.vector.tensor_copy(
                    map16[:, r * S + 512 * ch: r * S + 512 * ch + sz], pm[:, 0:sz]
                )

        # Output: 16 DMAs x 4.7MB, alternating the two HWDGE rings.
        src = map16.rearrange("p (a s) -> p a s", a=4)
        for g in range(16):
            dst = out[16 * g:16 * (g + 1)].rearrange("(a l) c s -> (l c) a s", a=4)
            eng = nc.sync if g % 2 == 0 else nc.scalar
            eng.dma_start(dst, src)

    return nc


_NC_CACHE: dict = {}


def _get_nc():
    if "nc" not in _NC_CACHE:
        nc = bacc.Bacc("TRN2", target_bir_lowering=False, debug=False,
                       num_devices=NCORES)
        _emit(nc)
        nc.compile()
        _NC_CACHE["nc"] = nc
    return _NC_CACHE["nc"]


def kernel(x, fc_w, fc_b, oh_w, oh_b, ot_w, ot_b, all_w, all_b):
    nc = _get_nc()
    xf = np.ascontiguousarray(np.asarray(x, dtype=np.float32).reshape(B, NF))
    params = {
        "fc_w": np.ascontiguousarray(fc_w, dtype=np.float32),
        "fc_b": np.ascontiguousarray(fc_b, dtype=np.float32),
        "oh_w": np.ascontiguousarray(oh_w, dtype=np.float32),
        "oh_b": np.ascontiguousarray(oh_b, dtype=np.float32),
        "ot_w": np.ascontiguousarray(ot_w, dtype=np.float32),
        "ot_b": np.ascontiguousarray(ot_b, dtype=np.float32),
        "all_w": np.ascontiguousarray(all_w, dtype=np.float32),
        "all_b": np.ascontiguousarray(all_b, dtype=np.float32),
    }
    in_maps = [
        {"x": np.ascontiguousarray(np.roll(xf, -BPC * k, axis=0)), **params}
        for k in range(NCORES)
    ]
    res = run_bass_kernel_spmd(nc, in_maps, list(range(NCORES)))
    full = np.empty((B, OUTC, H, W), dtype=np.float32)
    for k in range(NCORES):
        full[k::NCORES] = res.results[k]["out"].reshape(BPC, OUTC, H, W)
    return full
